# revision 42
# baseline (speedup 1.0000x reference)
"""Trainium2 Bass kernel: disparity regression via top-2 over the last axis.

pred[b, n] = sum_k topi_k * softmax(topv_k)  with K=2 over cost[b, n, :192].

Encoding: pack (quantized value, index) into one fp32 per element so a
single DVE max8 pass per row yields both top-2 values and indices:

    p = 256*q + (255-d),  q = round(8192*x)

q is produced by fp32 RNE at ulp 256: adding x*2^21 to an accumulator of
magnitude ~2^31 forces the quantization; (255-d) then rides exactly in the
low 8 bits. p is monotone in (quantized value, -d); equal quantized values
pick the lowest index (matching lax.top_k). Bias is 2^31+2^24 (bf16-exact;
keeps the whole |x|<=8 range inside the ulp-256 band [2^31, 2^32)).

Engine plan (v6): DVE's 512 max8 instructions (~133us) and DMA (~142us)
are the floor; the quantize+combine is spread so every other engine stays
near its measured hardware capacity. Per-tile routes, processed in
1536-column chunks:

  'G' (x7): ACT u = Id(x*2^21 + B24) (RNE quantize), ACT u2 = Id(u + C2)
      (= 256q + 2^21), GPSIMD tensor_add p = u2 + ((255-d) - 2^21).
  'T' (x6): TensorE, strict per-bank PSUM sequences (the PE group state
      machine does not tolerate interleaving): +B24 row (bf16 k=1),
      +SCALE*x (fp32r identity matmul - PSUM fp32 RNE quantizes; fp32r's
      ~tf32 truncation only perturbs values at ~1e-3, kept to 6/16 of
      the data), -B24 row, +(255-d) row; ACT copies PSUM->SBUF.
  'D' (x3): ACT u, then DVE affine_then_add p = (u + C2) + invt.

epilogue: pm = p-128 (DVE, tie-free), q256 = RNE256(pm) (DVE), d =
(q256+255) - p (DVE), s = sigmoid(256(q2-q1)/2^21) (ACT), diffs/product
on GPSIMD; pred = d1 + (d2-d1)*s. Results stage into one SBUF tile,
written out by two DMAs.
"""
import ml_dtypes
import numpy as np

import concourse.bacc as bacc
import concourse.tile as tile
import concourse.mybir as mybir
from concourse.bass_utils import run_bass_kernel_spmd

N_CORES = 8
B, N, D = 4, 131072, 192
ROWS = B * N                       # 524288
ROWS_PER_CORE = ROWS // N_CORES    # 65536
P = 128                            # SBUF partitions
G = 32                             # rows per partition per super-tile
TILE_ROWS = P * G                  # 4096
N_TILES = ROWS_PER_CORE // TILE_ROWS  # 16
COLS = G * D                       # 6144
CH_COLS = 1536                     # PSUM chunk: 8 rows = 3 banks
N_CH = COLS // CH_COLS             # 4
BK = 512                           # fp32 per PSUM bank
NQ = 2                             # DMA chunks per super-tile
EPG = 4                            # tiles per batched epilogue group
RPT = 768                          # period of the (255-d) bf16 row

F32 = mybir.dt.float32
F32R = mybir.dt.float32r
BF16 = mybir.dt.bfloat16
AF = mybir.ActivationFunctionType
OP = mybir.AluOpType

SCALE = float(2 ** 21)             # 8192 * 256
B24 = float(2 ** 31 + 2 ** 24)     # quantize bias, bf16-exact
INV_OFF = float(2 ** 21)
C2 = INV_OFF - B24

# per-tile routing: G=ACT+GPSIMD, T=TensorE(fp32r), D=ACT+DVE affine
ROUTE = "TGDTGGTGDTGGTGDT"


def build(loop_iters: int = 1, unroll: int = 1):
    nc = bacc.Bacc(
        "TRN2", target_bir_lowering=False, debug=False, num_devices=N_CORES
    )
    x = nc.dram_tensor("cost", [ROWS_PER_CORE, D], F32, kind="ExternalInput").ap()
    y = nc.dram_tensor("pred", [ROWS_PER_CORE], F32, kind="ExternalOutput").ap()

    x_t = x.rearrange("(t p g) d -> t p (g d)", p=P, g=G)
    # output staged in SBUF as [p, (t g)]; DRAM index = t*4096 + p*32 + g
    y_s = y.rearrange("(t p g) -> p t g", p=P, g=G)

    identS_d = nc.inline_tensor(
        np.eye(P, dtype=np.float32) * SCALE, name="identS"
    )
    dpat = np.arange(RPT, dtype=np.float64) % D
    invrow_d = nc.inline_tensor(
        (255.0 - dpat).astype(ml_dtypes.bfloat16).reshape(1, RPT),
        name="invrow"
    )
    dpat_ch = np.arange(CH_COLS, dtype=np.float64) % D
    invt_d = nc.inline_tensor(
        (255.0 - dpat_ch - INV_OFF).astype(np.float32).reshape(1, CH_COLS),
        name="invt"
    )

    def consts(tc, cp):
        biast = cp.tile([P, 1], F32)
        nc.gpsimd.memset(biast[:], B24)
        c2t = cp.tile([P, 1], F32)
        nc.gpsimd.memset(c2t[:], C2)
        identS = cp.tile([P, P], F32)
        nc.sync.dma_start(identS[:].bitcast(F32R),
                          identS_d.ap().bitcast(F32R))
        ones_b = cp.tile([1, P], BF16)
        nc.gpsimd.memset(ones_b[:], 1.0)
        rb = cp.tile([1, BK], BF16)
        nc.gpsimd.memset(rb[:], B24)
        nb = cp.tile([1, BK], BF16)
        nc.gpsimd.memset(nb[:], -B24)
        invrow = cp.tile([1, RPT], BF16)       # 255-d (bf16-exact)
        nc.sync.dma_start(invrow[:], invrow_d.ap())
        invt = cp.tile([P, CH_COLS], F32)      # (255-d) - 2^21, one chunk
        nc.sync.dma_start(invt[:], invt_d.ap().partition_broadcast(P))
        c128g = cp.tile([P, 2 * G], F32)       # epilogue consts for POOL
        nc.gpsimd.memset(c128g[:], 128.0)
        b24g = cp.tile([P, 2 * G], F32)
        nc.gpsimd.memset(b24g[:], B24)
        c255g = cp.tile([P, 2 * G], F32)
        nc.gpsimd.memset(c255g[:], 255.0)
        pa = cp.tile([P, N_TILES * G], F32)    # staged predictions
        return (biast, c2t, identS, ones_b, rb, nb, invrow, invt,
                c128g, b24g, c255g, pa)

    def body(tc, cn, xp, up, pp, qp, vp, ep):
        (biast, c2t, identS, ones_b, rb, nb, invrow, invt,
         c128g, b24g, c255g, pa) = cn
        xp, xpt = xp
        for t in range(N_TILES):
            route = ROUTE[t]
            # separate pools: only T tiles carry the f32r tag (walrus'
            # rounded-producer rule); G/D tiles load exact f32
            xt = (xpt if route == "T" else xp).tile([P, COLS], F32)
            for q in range(NQ):
                c0, c1 = q * (COLS // NQ), (q + 1) * (COLS // NQ)
                if route == "T":
                    nc.sync.dma_start(xt[:, c0:c1].bitcast(F32R),
                                      x_t[t][:, c0:c1].bitcast(F32R))
                else:
                    nc.sync.dma_start(xt[:, c0:c1], x_t[t][:, c0:c1])

            if t % EPG == 0:
                vg = vp.tile([P, EPG * G * 8], F32)
            v8 = vg[:, (t % EPG) * G * 8:(t % EPG + 1) * G * 8]
            xr = xt[:].bitcast(F32R)
            identSr = identS[:].bitcast(F32R)
            for ch in range(N_CH):
                lo = ch * CH_COLS
                hi = lo + CH_COLS
                pk = pp.tile([P, CH_COLS], F32)
                if route == "T":
                    ps = qp.tile([P, CH_COLS], F32)
                    # strict per-bank accumulation sequences: the PE's
                    # group state machine does not tolerate interleaving
                    for b in range(CH_COLS // BK):
                        dst = ps[:, b * BK:(b + 1) * BK]
                        col = lo + b * BK
                        off = col % D
                        nc.tensor.matmul(dst, ones_b[:], rb[:],
                                         start=True, stop=False)
                        nc.tensor.matmul(dst, identSr,
                                         xr[:, col:col + BK],
                                         start=False, stop=False)
                        nc.tensor.matmul(dst, ones_b[:], nb[:],
                                         start=False, stop=False)
                        nc.tensor.matmul(dst, ones_b[:],
                                         invrow[:, off:off + BK],
                                         start=False, stop=True)
                    nc.scalar.activation(pk[:], ps[:], AF.Identity)
                elif route == "G":
                    ut = up.tile([P, CH_COLS], F32)
                    nc.scalar.activation(ut[:], xt[:, lo:hi], AF.Identity,
                                         bias=biast[:], scale=SCALE)
                    nc.scalar.activation(ut[:], ut[:], AF.Identity,
                                         bias=c2t[:])
                    nc.gpsimd.tensor_add(pk[:], ut[:], invt[:])
                else:
                    ut = up.tile([P, CH_COLS], F32)
                    nc.scalar.activation(ut[:], xt[:, lo:hi], AF.Identity,
                                         bias=biast[:], scale=SCALE)
                    nc.vector.affine_then_add(pk[:], ut[:], invt[:],
                                              scale=1.0, bias=C2)
                for g in range(CH_COLS // D):
                    r = ch * (CH_COLS // D) + g
                    nc.vector.max(v8[:, r * 8:(r + 1) * 8],
                                  pk[:, g * D:(g + 1) * D])

            if t % EPG == EPG - 1:
                # batched epilogue over EPG tiles' worth of rows (cuts the
                # per-op q7 launch/join count 4x)
                GW = EPG * G
                v8v = vg[:].rearrange("p (g k) -> p g k", k=8)
                pair = v8v[:, :, 0:2]                 # [P, GW, 2]

                # inv = p mod 256 via a tie-free fp32 round-trip: pm =
                # p-128 (inv-128 in [-64,127], never a .5 tie), q256 =
                # RNE-to-256(pm) = 256*q, d = (q256+255) - p.  All exact.
                pm = ep.tile([P, 2 * GW], F32)
                pmv = pm[:].rearrange("p (g k) -> p g k", k=2)
                nc.vector.tensor_single_scalar(pmv, pair, -128.0, OP.add)
                q256 = ep.tile([P, 2 * GW], F32)
                nc.vector.tensor_scalar(q256[:], pm[:], B24, B24,
                                        OP.add, OP.subtract)
                q_v = q256[:].rearrange("p (g k) -> p g k", k=2)
                dd = ep.tile([P, 2 * GW], F32)        # d = (q256+255) - p
                dd_v = dd[:].rearrange("p (g k) -> p g k", k=2)
                nc.vector.scalar_tensor_tensor(dd_v, q_v, 255.0, pair,
                                               OP.add, OP.subtract)

                sm = ep.tile([P, GW], F32)            # 256*(q2-q1) <= 0
                nc.gpsimd.tensor_sub(sm[:], q_v[:, :, 1], q_v[:, :, 0])
                invd = ep.tile([P, GW], F32)          # d2-d1
                nc.gpsimd.tensor_sub(invd[:], dd_v[:, :, 1], dd_v[:, :, 0])
                s = ep.tile([P, GW], F32)
                nc.scalar.activation(s[:], sm[:], AF.Sigmoid,
                                     scale=1.0 / SCALE)
                w = ep.tile([P, GW], F32)
                nc.gpsimd.tensor_mul(w[:], invd[:], s[:])
                # pred = d1 + (d2-d1)*s, staged for the batched output
                nc.gpsimd.tensor_add(pa[:, (t + 1 - EPG) * G:(t + 1) * G],
                                     dd_v[:, :, 0], w[:])

            if t == N_TILES // 2 - 1 or t == N_TILES - 1:
                h0 = 0 if t < N_TILES // 2 else N_TILES // 2
                pav = pa[:, h0 * G:(t + 1) * G].rearrange(
                    "p (t g) -> p t g", g=G)
                nc.sync.dma_start(y_s[:, h0:t + 1], pav)

    with tile.TileContext(nc) as tc:
        with (
            tc.tile_pool(name="cp", bufs=1) as cp,
            tc.tile_pool(name="xp", bufs=3) as xp,
            tc.tile_pool(name="xpt", bufs=2) as xpt,
            tc.tile_pool(name="up", bufs=4) as up,
            tc.tile_pool(name="pp", bufs=6) as pp,
            tc.tile_pool(name="qp", bufs=2, space="PSUM") as qp,
            tc.tile_pool(name="vp", bufs=2) as vp,
            tc.tile_pool(name="ep", bufs=1) as ep,
        ):
            cn = consts(tc, cp)
            if loop_iters == 1:
                for _ in range(unroll):
                    body(tc, cn, (xp, xpt), up, pp, qp, vp, ep)
            else:
                with tc.For_i(0, loop_iters, 1):
                    body(tc, cn, (xp, xpt), up, pp, qp, vp, ep)

    nc.compile()
    return nc


_NC_CACHE = {}


def _get_nc(loop_iters: int = 1):
    if loop_iters not in _NC_CACHE:
        _NC_CACHE[loop_iters] = build(loop_iters)
    return _NC_CACHE[loop_iters]


def run(cost: np.ndarray, loop_iters: int = 1) -> np.ndarray:
    nc = _get_nc(loop_iters)
    flat = np.ascontiguousarray(cost.reshape(ROWS, D))
    in_maps = [
        {"cost": flat[c * ROWS_PER_CORE:(c + 1) * ROWS_PER_CORE]}
        for c in range(N_CORES)
    ]
    res = run_bass_kernel_spmd(nc, in_maps, core_ids=list(range(N_CORES)))
    out = np.concatenate(
        [res.results[c]["pred"] for c in range(N_CORES)]
    )
    return out.reshape(B, N).astype(np.float32, copy=False)


def kernel(cost: np.ndarray) -> np.ndarray:
    return run(cost, loop_iters=1)


# revision 43
# speedup vs baseline: 1.1271x; 1.1271x over previous
"""Trainium2 Bass kernel: disparity regression via top-2 over the last axis.

pred[b, n] = sum_k topi_k * softmax(topv_k)  with K=2 over cost[b, n, :192].

Encoding: pack (quantized value, index) into one fp32 per element so a
single DVE max8 pass per row yields both top-2 values and indices:

    p = 256*q + (255-d),  q = round(8192*x)

q is produced by fp32 RNE at ulp 256: adding x*2^21 to an accumulator of
magnitude ~2^31 forces the quantization; (255-d) then rides exactly in the
low 8 bits. p is monotone in (quantized value, -d); equal quantized values
pick the lowest index (matching lax.top_k). Bias is 2^31+2^24 (bf16-exact;
keeps the whole |x|<=8 range inside the ulp-256 band [2^31, 2^32)).

Engine plan (v6): DVE's 512 max8 instructions (~133us) and DMA (~142us)
are the floor; the quantize+combine is spread so every other engine stays
near its measured hardware capacity. Per-tile routes, processed in
1536-column chunks:

  'G' (x7): ACT u = Id(x*2^21 + B24) (RNE quantize), ACT u2 = Id(u + C2)
      (= 256q + 2^21), GPSIMD tensor_add p = u2 + ((255-d) - 2^21).
  'T' (x6): TensorE, strict per-bank PSUM sequences (the PE group state
      machine does not tolerate interleaving): +B24 row (bf16 k=1),
      +SCALE*x (fp32r identity matmul - PSUM fp32 RNE quantizes; fp32r's
      ~tf32 truncation only perturbs values at ~1e-3, kept to 6/16 of
      the data), -B24 row, +(255-d) row; ACT copies PSUM->SBUF.
  'D' (x3): ACT u, then DVE affine_then_add p = (u + C2) + invt.

epilogue: pm = p-128 (DVE, tie-free), q256 = RNE256(pm) (DVE), d =
(q256+255) - p (DVE), s = sigmoid(256(q2-q1)/2^21) (ACT), diffs/product
on GPSIMD; pred = d1 + (d2-d1)*s. Results stage into one SBUF tile,
written out by two DMAs.
"""
import ml_dtypes
import numpy as np

import concourse.bacc as bacc
import concourse.tile as tile
import concourse.mybir as mybir
from concourse.bass_utils import run_bass_kernel_spmd

N_CORES = 8
B, N, D = 4, 131072, 192
ROWS = B * N                       # 524288
ROWS_PER_CORE = ROWS // N_CORES    # 65536
P = 128                            # SBUF partitions
G = 32                             # rows per partition per super-tile
TILE_ROWS = P * G                  # 4096
N_TILES = ROWS_PER_CORE // TILE_ROWS  # 16
COLS = G * D                       # 6144
CH_COLS = 1536                     # PSUM chunk: 8 rows = 3 banks
N_CH = COLS // CH_COLS             # 4
BK = 512                           # fp32 per PSUM bank
NQ = 2                             # DMA chunks per super-tile
RPT = 768                          # period of the (255-d) bf16 row

F32 = mybir.dt.float32
F32R = mybir.dt.float32r
BF16 = mybir.dt.bfloat16
AF = mybir.ActivationFunctionType
OP = mybir.AluOpType

SCALE = float(2 ** 21)             # 8192 * 256
B24 = float(2 ** 31 + 2 ** 24)     # quantize bias, bf16-exact
INV_OFF = float(2 ** 21)
C2 = INV_OFF - B24

# per-tile routing: G=ACT+GPSIMD, T=TensorE(fp32r), D=ACT+DVE affine
ROUTE = "TGDTGGTGDTGGTGDT"


def build(loop_iters: int = 1, unroll: int = 1):
    nc = bacc.Bacc(
        "TRN2", target_bir_lowering=False, debug=False, num_devices=N_CORES
    )
    x = nc.dram_tensor("cost", [ROWS_PER_CORE, D], F32, kind="ExternalInput").ap()
    y = nc.dram_tensor("pred", [ROWS_PER_CORE], F32, kind="ExternalOutput").ap()

    x_t = x.rearrange("(t p g) d -> t p (g d)", p=P, g=G)
    # output staged in SBUF as [p, (t g)]; DRAM index = t*4096 + p*32 + g
    y_s = y.rearrange("(t p g) -> p t g", p=P, g=G)

    identS_d = nc.inline_tensor(
        np.eye(P, dtype=np.float32) * SCALE, name="identS"
    )
    dpat = np.arange(RPT, dtype=np.float64) % D
    invrow_d = nc.inline_tensor(
        (255.0 - dpat).astype(ml_dtypes.bfloat16).reshape(1, RPT),
        name="invrow"
    )
    dpat_ch = np.arange(CH_COLS, dtype=np.float64) % D
    invt_d = nc.inline_tensor(
        (255.0 - dpat_ch - INV_OFF).astype(np.float32).reshape(1, CH_COLS),
        name="invt"
    )

    def consts(tc, cp):
        biast = cp.tile([P, 1], F32)
        nc.gpsimd.memset(biast[:], B24)
        c2t = cp.tile([P, 1], F32)
        nc.gpsimd.memset(c2t[:], C2)
        identS = cp.tile([P, P], F32)
        nc.sync.dma_start(identS[:].bitcast(F32R),
                          identS_d.ap().bitcast(F32R))
        ones_b = cp.tile([1, P], BF16)
        nc.gpsimd.memset(ones_b[:], 1.0)
        rb = cp.tile([1, BK], BF16)
        nc.gpsimd.memset(rb[:], B24)
        nb = cp.tile([1, BK], BF16)
        nc.gpsimd.memset(nb[:], -B24)
        invrow = cp.tile([1, RPT], BF16)       # 255-d (bf16-exact)
        nc.sync.dma_start(invrow[:], invrow_d.ap())
        invt = cp.tile([P, CH_COLS], F32)      # (255-d) - 2^21, one chunk
        nc.sync.dma_start(invt[:], invt_d.ap().partition_broadcast(P))
        pa = cp.tile([P, N_TILES * G], F32)    # staged predictions
        return biast, c2t, identS, ones_b, rb, nb, invrow, invt, pa

    def body(tc, cn, xp, up, pp, qp, vp, ep):
        biast, c2t, identS, ones_b, rb, nb, invrow, invt, pa = cn
        xp, xpt = xp
        for t in range(N_TILES):
            route = ROUTE[t]
            # separate pools: only T tiles carry the f32r tag (walrus'
            # rounded-producer rule); G/D tiles load exact f32
            xt = (xpt if route == "T" else xp).tile([P, COLS], F32)
            for q in range(NQ):
                c0, c1 = q * (COLS // NQ), (q + 1) * (COLS // NQ)
                if route == "T":
                    nc.sync.dma_start(xt[:, c0:c1].bitcast(F32R),
                                      x_t[t][:, c0:c1].bitcast(F32R))
                else:
                    nc.sync.dma_start(xt[:, c0:c1], x_t[t][:, c0:c1])

            v8 = vp.tile([P, G * 8], F32)
            xr = xt[:].bitcast(F32R)
            identSr = identS[:].bitcast(F32R)
            for ch in range(N_CH):
                lo = ch * CH_COLS
                hi = lo + CH_COLS
                pk = pp.tile([P, CH_COLS], F32)
                if route == "T":
                    ps = qp.tile([P, CH_COLS], F32)
                    # strict per-bank accumulation sequences: the PE's
                    # group state machine does not tolerate interleaving
                    for b in range(CH_COLS // BK):
                        dst = ps[:, b * BK:(b + 1) * BK]
                        col = lo + b * BK
                        off = col % D
                        nc.tensor.matmul(dst, ones_b[:], rb[:],
                                         start=True, stop=False)
                        nc.tensor.matmul(dst, identSr,
                                         xr[:, col:col + BK],
                                         start=False, stop=False)
                        nc.tensor.matmul(dst, ones_b[:], nb[:],
                                         start=False, stop=False)
                        nc.tensor.matmul(dst, ones_b[:],
                                         invrow[:, off:off + BK],
                                         start=False, stop=True)
                    nc.scalar.activation(pk[:], ps[:], AF.Identity)
                elif route == "G":
                    ut = up.tile([P, CH_COLS], F32)
                    nc.scalar.activation(ut[:], xt[:, lo:hi], AF.Identity,
                                         bias=biast[:], scale=SCALE)
                    nc.scalar.activation(ut[:], ut[:], AF.Identity,
                                         bias=c2t[:])
                    nc.gpsimd.tensor_add(pk[:], ut[:], invt[:])
                else:
                    ut = up.tile([P, CH_COLS], F32)
                    nc.scalar.activation(ut[:], xt[:, lo:hi], AF.Identity,
                                         bias=biast[:], scale=SCALE)
                    nc.vector.affine_then_add(pk[:], ut[:], invt[:],
                                              scale=1.0, bias=C2)
                for g in range(CH_COLS // D):
                    r = ch * (CH_COLS // D) + g
                    nc.vector.max(v8[:, r * 8:(r + 1) * 8],
                                  pk[:, g * D:(g + 1) * D])

            v8v = v8[:].rearrange("p (g k) -> p g k", k=8)
            pair = v8v[:, :, 0:2]                     # [P, G, 2]

            # inv = p mod 256 via a tie-free fp32 round-trip: pm = p-128
            # (inv-128 in [-64,127], never a .5 tie), q256 = RNE-to-256(pm)
            # = 256*q, d = (q256+255) - p.  All exact in fp32.
            pm = ep.tile([P, 2 * G], F32)
            pmv = pm[:].rearrange("p (g k) -> p g k", k=2)
            nc.vector.tensor_single_scalar(pmv, pair, -128.0, OP.add)
            q256 = ep.tile([P, 2 * G], F32)
            nc.vector.tensor_scalar(q256[:], pm[:], B24, B24,
                                    OP.add, OP.subtract)
            q_v = q256[:].rearrange("p (g k) -> p g k", k=2)
            dd = ep.tile([P, 2 * G], F32)             # d = (q256+255) - p
            dd_v = dd[:].rearrange("p (g k) -> p g k", k=2)
            nc.vector.scalar_tensor_tensor(dd_v, q_v, 255.0, pair,
                                           OP.add, OP.subtract)

            sm = ep.tile([P, G], F32)                 # 256*(q2-q1) <= 0
            nc.gpsimd.tensor_sub(sm[:], q_v[:, :, 1], q_v[:, :, 0])
            invd = ep.tile([P, G], F32)               # d2-d1
            nc.gpsimd.tensor_sub(invd[:], dd_v[:, :, 1], dd_v[:, :, 0])
            s = ep.tile([P, G], F32)
            nc.scalar.activation(s[:], sm[:], AF.Sigmoid, scale=1.0 / SCALE)
            w = ep.tile([P, G], F32)
            nc.gpsimd.tensor_mul(w[:], invd[:], s[:])
            # pred = d1 + (d2-d1)*s, staged into the batched output tile
            nc.gpsimd.tensor_add(pa[:, t * G:(t + 1) * G], dd_v[:, :, 0],
                                 w[:])

            if t == N_TILES // 2 - 1 or t == N_TILES - 1:
                h0 = 0 if t < N_TILES // 2 else N_TILES // 2
                pav = pa[:, h0 * G:(t + 1) * G].rearrange(
                    "p (t g) -> p t g", g=G)
                nc.sync.dma_start(y_s[:, h0:t + 1], pav)

    with tile.TileContext(nc) as tc:
        with (
            tc.tile_pool(name="cp", bufs=1) as cp,
            tc.tile_pool(name="xp", bufs=3) as xp,
            tc.tile_pool(name="xpt", bufs=2) as xpt,
            tc.tile_pool(name="up", bufs=4) as up,
            tc.tile_pool(name="pp", bufs=6) as pp,
            tc.tile_pool(name="qp", bufs=2, space="PSUM") as qp,
            tc.tile_pool(name="vp", bufs=4) as vp,
            tc.tile_pool(name="ep", bufs=4) as ep,
        ):
            cn = consts(tc, cp)
            if loop_iters == 1:
                for _ in range(unroll):
                    body(tc, cn, (xp, xpt), up, pp, qp, vp, ep)
            else:
                with tc.For_i(0, loop_iters, 1):
                    body(tc, cn, (xp, xpt), up, pp, qp, vp, ep)

    nc.compile()
    return nc


_NC_CACHE = {}


def _get_nc(loop_iters: int = 1):
    if loop_iters not in _NC_CACHE:
        _NC_CACHE[loop_iters] = build(loop_iters)
    return _NC_CACHE[loop_iters]


def run(cost: np.ndarray, loop_iters: int = 1) -> np.ndarray:
    nc = _get_nc(loop_iters)
    flat = np.ascontiguousarray(cost.reshape(ROWS, D))
    in_maps = [
        {"cost": flat[c * ROWS_PER_CORE:(c + 1) * ROWS_PER_CORE]}
        for c in range(N_CORES)
    ]
    res = run_bass_kernel_spmd(nc, in_maps, core_ids=list(range(N_CORES)))
    out = np.concatenate(
        [res.results[c]["pred"] for c in range(N_CORES)]
    )
    return out.reshape(B, N).astype(np.float32, copy=False)


def kernel(cost: np.ndarray) -> np.ndarray:
    return run(cost, loop_iters=1)


# revision 48
# speedup vs baseline: 1.1355x; 1.0075x over previous
"""Trainium2 Bass kernel: disparity regression via top-2 over the last axis.

pred[b, n] = sum_k topi_k * softmax(topv_k)  with K=2 over cost[b, n, :192].

Encoding: pack (quantized value, index) into one fp32 per element so a
single DVE max8 pass per row yields both top-2 values and indices:

    p = 256*q + (255-d),  q = round(8192*x)

q is produced by fp32 RNE at ulp 256: adding x*2^21 to an accumulator of
magnitude ~2^31 forces the quantization; (255-d) then rides exactly in the
low 8 bits. p is monotone in (quantized value, -d); equal quantized values
pick the lowest index (matching lax.top_k). Bias is 2^31+2^24 (bf16-exact;
keeps the whole |x|<=8 range inside the ulp-256 band [2^31, 2^32)).

Engine plan (v6): DVE's 512 max8 instructions (~133us) and DMA (~142us)
are the floor; the quantize+combine is spread so every other engine stays
near its measured hardware capacity. Per-tile routes, processed in
1536-column chunks:

  'G' (x7): ACT u = Id(x*2^21 + B24) (RNE quantize), ACT u2 = Id(u + C2)
      (= 256q + 2^21), GPSIMD tensor_add p = u2 + ((255-d) - 2^21).
  'T' (x6): TensorE, strict per-bank PSUM sequences (the PE group state
      machine does not tolerate interleaving): +B24 row (bf16 k=1),
      +SCALE*x (fp32r identity matmul - PSUM fp32 RNE quantizes; fp32r's
      ~tf32 truncation only perturbs values at ~1e-3, kept to 6/16 of
      the data), -B24 row, +(255-d) row; ACT copies PSUM->SBUF.
  'D' (x3): ACT u, then DVE affine_then_add p = (u + C2) + invt.

epilogue: pm = p-128 (DVE, tie-free), q256 = RNE256(pm) (DVE), d =
(q256+255) - p (DVE), s = sigmoid(256(q2-q1)/2^21) (ACT), diffs/product
on GPSIMD; pred = d1 + (d2-d1)*s. Results stage into one SBUF tile,
written out by two DMAs.
"""
import ml_dtypes
import numpy as np

import concourse.bacc as bacc
import concourse.tile as tile
import concourse.mybir as mybir
from concourse.bass_utils import run_bass_kernel_spmd

N_CORES = 8
B, N, D = 4, 131072, 192
ROWS = B * N                       # 524288
ROWS_PER_CORE = ROWS // N_CORES    # 65536
P = 128                            # SBUF partitions
G = 32                             # rows per partition per super-tile
TILE_ROWS = P * G                  # 4096
N_TILES = ROWS_PER_CORE // TILE_ROWS  # 16
COLS = G * D                       # 6144
CH_COLS = 1536                     # PSUM chunk: 8 rows = 3 banks
N_CH = COLS // CH_COLS             # 4
BK = 512                           # fp32 per PSUM bank
NQ = 2                             # DMA chunks per super-tile
EPG = 4                            # tiles per batched epilogue group
RPT = 768                          # period of the (255-d) bf16 row

F32 = mybir.dt.float32
F32R = mybir.dt.float32r
BF16 = mybir.dt.bfloat16
AF = mybir.ActivationFunctionType
OP = mybir.AluOpType

SCALE = float(2 ** 21)             # 8192 * 256
B24 = float(2 ** 31 + 2 ** 24)     # quantize bias, bf16-exact
INV_OFF = float(2 ** 21)
C2 = INV_OFF - B24

# per-tile routing: G=ACT+GPSIMD, T=TensorE(fp32r), D=ACT+DVE affine
ROUTE = "TGDTGGTGDTGGTGDT"


def build(loop_iters: int = 1, unroll: int = 1):
    nc = bacc.Bacc(
        "TRN2", target_bir_lowering=False, debug=False, num_devices=N_CORES
    )
    x = nc.dram_tensor("cost", [ROWS_PER_CORE, D], F32, kind="ExternalInput").ap()
    y = nc.dram_tensor("pred", [ROWS_PER_CORE], F32, kind="ExternalOutput").ap()

    x_t = x.rearrange("(t p g) d -> t p (g d)", p=P, g=G)
    # output staged in SBUF as [p, (t g)]; DRAM index = t*4096 + p*32 + g
    y_s = y.rearrange("(t p g) -> p t g", p=P, g=G)

    identS_d = nc.inline_tensor(
        np.eye(P, dtype=np.float32) * SCALE, name="identS"
    )
    dpat = np.arange(RPT, dtype=np.float64) % D
    invrow_d = nc.inline_tensor(
        (255.0 - dpat).astype(ml_dtypes.bfloat16).reshape(1, RPT),
        name="invrow"
    )
    dpat_ch = np.arange(CH_COLS, dtype=np.float64) % D
    invt_d = nc.inline_tensor(
        (255.0 - dpat_ch - INV_OFF).astype(np.float32).reshape(1, CH_COLS),
        name="invt"
    )

    def consts(tc, cp):
        biast = cp.tile([P, 1], F32)
        nc.gpsimd.memset(biast[:], B24)
        c2t = cp.tile([P, 1], F32)
        nc.gpsimd.memset(c2t[:], C2)
        identS = cp.tile([P, P], F32)
        nc.sync.dma_start(identS[:].bitcast(F32R),
                          identS_d.ap().bitcast(F32R))
        ones_b = cp.tile([1, P], BF16)
        nc.gpsimd.memset(ones_b[:], 1.0)
        rb = cp.tile([1, BK], BF16)
        nc.gpsimd.memset(rb[:], B24)
        nb = cp.tile([1, BK], BF16)
        nc.gpsimd.memset(nb[:], -B24)
        invrow = cp.tile([1, RPT], BF16)       # 255-d (bf16-exact)
        nc.sync.dma_start(invrow[:], invrow_d.ap())
        invt = cp.tile([P, CH_COLS], F32)      # (255-d) - 2^21, one chunk
        nc.sync.dma_start(invt[:], invt_d.ap().partition_broadcast(P))
        pa = cp.tile([P, N_TILES * G], F32)    # staged predictions
        return biast, c2t, identS, ones_b, rb, nb, invrow, invt, pa

    def body(tc, cn, xp, up, pp, qp, vp, ep):
        biast, c2t, identS, ones_b, rb, nb, invrow, invt, pa = cn
        xp, xpt = xp
        for t in range(N_TILES):
            route = ROUTE[t]
            # separate pools: only T tiles carry the f32r tag (walrus'
            # rounded-producer rule); G/D tiles load exact f32
            xt = (xpt if route == "T" else xp).tile([P, COLS], F32)
            for q in range(NQ):
                c0, c1 = q * (COLS // NQ), (q + 1) * (COLS // NQ)
                if route == "T":
                    nc.sync.dma_start(xt[:, c0:c1].bitcast(F32R),
                                      x_t[t][:, c0:c1].bitcast(F32R))
                else:
                    nc.sync.dma_start(xt[:, c0:c1], x_t[t][:, c0:c1])

            if t % EPG == 0:
                vg = vp.tile([P, EPG * G * 8], F32)
            v8 = vg[:, (t % EPG) * G * 8:(t % EPG + 1) * G * 8]
            xr = xt[:].bitcast(F32R)
            identSr = identS[:].bitcast(F32R)
            for ch in range(N_CH):
                lo = ch * CH_COLS
                hi = lo + CH_COLS
                pk = pp.tile([P, CH_COLS], F32)
                if route == "T":
                    ps = qp.tile([P, CH_COLS], F32)
                    # strict per-bank accumulation sequences: the PE's
                    # group state machine does not tolerate interleaving
                    for b in range(CH_COLS // BK):
                        dst = ps[:, b * BK:(b + 1) * BK]
                        col = lo + b * BK
                        off = col % D
                        nc.tensor.matmul(dst, ones_b[:], rb[:],
                                         start=True, stop=False)
                        nc.tensor.matmul(dst, identSr,
                                         xr[:, col:col + BK],
                                         start=False, stop=False)
                        nc.tensor.matmul(dst, ones_b[:], nb[:],
                                         start=False, stop=False)
                        nc.tensor.matmul(dst, ones_b[:],
                                         invrow[:, off:off + BK],
                                         start=False, stop=True)
                    nc.scalar.activation(pk[:], ps[:], AF.Identity)
                elif route == "G":
                    ut = up.tile([P, CH_COLS], F32)
                    nc.scalar.activation(ut[:], xt[:, lo:hi], AF.Identity,
                                         bias=biast[:], scale=SCALE)
                    nc.scalar.activation(ut[:], ut[:], AF.Identity,
                                         bias=c2t[:])
                    nc.gpsimd.tensor_add(pk[:], ut[:], invt[:])
                else:
                    ut = up.tile([P, CH_COLS], F32)
                    nc.scalar.activation(ut[:], xt[:, lo:hi], AF.Identity,
                                         bias=biast[:], scale=SCALE)
                    nc.vector.affine_then_add(pk[:], ut[:], invt[:],
                                              scale=1.0, bias=C2)
                for g in range(CH_COLS // D):
                    r = ch * (CH_COLS // D) + g
                    nc.vector.max(v8[:, r * 8:(r + 1) * 8],
                                  pk[:, g * D:(g + 1) * D])

            if t % EPG == EPG - 1:
                # batched epilogue over EPG tiles' worth of rows (cuts the
                # per-op q7 launch/join count 4x; ep double-buffered so
                # consecutive groups overlap)
                GW = EPG * G
                v8v = vg[:].rearrange("p (g k) -> p g k", k=8)
                pair = v8v[:, :, 0:2]                 # [P, GW, 2]

                # inv = p mod 256 via a tie-free fp32 round-trip: pm =
                # p-128 (inv-128 in [-64,127], never a .5 tie), q256 =
                # RNE-to-256(pm) = 256*q, d = (q256+255) - p.  All exact.
                pm = ep.tile([P, 2 * GW], F32)
                pmv = pm[:].rearrange("p (g k) -> p g k", k=2)
                nc.vector.tensor_single_scalar(pmv, pair, -128.0, OP.add)
                q256 = ep.tile([P, 2 * GW], F32)
                nc.vector.tensor_scalar(q256[:], pm[:], B24, B24,
                                        OP.add, OP.subtract)
                q_v = q256[:].rearrange("p (g k) -> p g k", k=2)
                dd = ep.tile([P, 2 * GW], F32)        # d = (q256+255) - p
                dd_v = dd[:].rearrange("p (g k) -> p g k", k=2)
                nc.vector.scalar_tensor_tensor(dd_v, q_v, 255.0, pair,
                                               OP.add, OP.subtract)

                sm = ep.tile([P, GW], F32)            # 256*(q2-q1) <= 0
                nc.gpsimd.tensor_sub(sm[:], q_v[:, :, 1], q_v[:, :, 0])
                invd = ep.tile([P, GW], F32)          # d2-d1
                nc.gpsimd.tensor_sub(invd[:], dd_v[:, :, 1], dd_v[:, :, 0])
                s = ep.tile([P, GW], F32)
                nc.scalar.activation(s[:], sm[:], AF.Sigmoid,
                                     scale=1.0 / SCALE)
                w = ep.tile([P, GW], F32)
                nc.gpsimd.tensor_mul(w[:], invd[:], s[:])
                # pred = d1 + (d2-d1)*s, staged for the batched output
                nc.gpsimd.tensor_add(pa[:, (t + 1 - EPG) * G:(t + 1) * G],
                                     dd_v[:, :, 0], w[:])

            if t == N_TILES // 2 - 1 or t == N_TILES - 1:
                h0 = 0 if t < N_TILES // 2 else N_TILES // 2
                pav = pa[:, h0 * G:(t + 1) * G].rearrange(
                    "p (t g) -> p t g", g=G)
                nc.sync.dma_start(y_s[:, h0:t + 1], pav)

    with tile.TileContext(nc) as tc:
        with (
            tc.tile_pool(name="cp", bufs=1) as cp,
            tc.tile_pool(name="xp", bufs=3) as xp,
            tc.tile_pool(name="xpt", bufs=2) as xpt,
            tc.tile_pool(name="up", bufs=3) as up,
            tc.tile_pool(name="pp", bufs=6) as pp,
            tc.tile_pool(name="qp", bufs=2, space="PSUM") as qp,
            tc.tile_pool(name="vp", bufs=2) as vp,
            tc.tile_pool(name="ep", bufs=2) as ep,
        ):
            cn = consts(tc, cp)
            if loop_iters == 1:
                for _ in range(unroll):
                    body(tc, cn, (xp, xpt), up, pp, qp, vp, ep)
            else:
                with tc.For_i(0, loop_iters, 1):
                    body(tc, cn, (xp, xpt), up, pp, qp, vp, ep)

    nc.compile()
    return nc


_NC_CACHE = {}


def _get_nc(loop_iters: int = 1):
    if loop_iters not in _NC_CACHE:
        _NC_CACHE[loop_iters] = build(loop_iters)
    return _NC_CACHE[loop_iters]


def run(cost: np.ndarray, loop_iters: int = 1) -> np.ndarray:
    nc = _get_nc(loop_iters)
    flat = np.ascontiguousarray(cost.reshape(ROWS, D))
    in_maps = [
        {"cost": flat[c * ROWS_PER_CORE:(c + 1) * ROWS_PER_CORE]}
        for c in range(N_CORES)
    ]
    res = run_bass_kernel_spmd(nc, in_maps, core_ids=list(range(N_CORES)))
    out = np.concatenate(
        [res.results[c]["pred"] for c in range(N_CORES)]
    )
    return out.reshape(B, N).astype(np.float32, copy=False)


def kernel(cost: np.ndarray) -> np.ndarray:
    return run(cost, loop_iters=1)
